# revision 18
# baseline (speedup 1.0000x reference)
"""GCN (2-layer, PyG GCNConv semantics) on 8 Trainium2 NeuronCores.

Strategy (v7)
-------------
Host does layout + O(N) normalization; each NeuronCore reduces dense
bf16 message grids. See v3 notes: layer-1 grid holds x'(u)=dinv(u)*x(u)
(+ self slot); layer-2 messages y1'(u)=dinv(u)*y1(u) are sign-split into
positive/negative grids so s+/s- are plain row-sums (b1 == 0, C_in == 1);
out[v,j] = dinv_v*(a_j*s+ + c_j*s-) + b2_j is O(N) host work.

Row-sums are computed two ways, picked per degree class:
 - DVE path: [128, rows, S] row-major layout; bf16 pairwise-add halving
   (DVE 2x mode) then a 1x tensor_reduce.
 - PE path (classes where k=floor(128/S) wastes <=7% of partitions):
   slot-major layout [S*lane + s, node//k]; row-sum = ones-blockdiag
   [128,k] matmul on the otherwise-idle tensor engine, accumulated in
   PSUM (4 banks per group) and DMA'd straight to DRAM.

Nodes are width-sorted and dealt round-robin to the 8 cores per grid, so
all cores share one SPMD NEFF and work balances to <0.1%; no collectives.
"""
import sys

sys.path.insert(0, "/opt/trn_rl_repo")

import numpy as np

N_CORES = 8
CLS_STEP = 4            # slot-count class granularity
CHUNK_COLS = 4608       # <=9.2KB/partition bf16 per DMA chunk
HALVE_MIN_COLS = 768    # min piece cols for the 2x pairwise-add pass
HALVE2_MIN_COLS = 3072  # min cols for a second halving level (S%8==0)
FIRST_CHUNK_COLS = 384  # tiny lead chunk so the DVE starts ASAP
PE_WASTE_MAX = 0.07     # max idle-partition fraction for the PE path
PE_MIN_COLS = 384       # min class cols for the PE path
MM = 512                # matmul moving-operand max free dim
PE_GROUP = 4            # matmuls (PSUM banks) per PSUM tile / out DMA
KMAX = 16               # partition dim of the PE output DRAM tensor

_NEFF_CACHE: dict = {}


class _Section:
    """Packing of one grid section (one slot-count distribution)."""

    def __init__(self, slot_counts):
        active = np.flatnonzero(slot_counts > 0)
        sc = slot_counts[active]
        order = np.argsort(sc, kind="stable")
        nodes = active[order]                   # width-sorted node ids
        widths = sc[order]
        clsS = (CLS_STEP * np.ceil(widths / CLS_STEP)).astype(np.int64)
        Svals, starts, cnts = np.unique(clsS, return_index=True,
                                        return_counts=True)
        # per class: kind k>0 => PE with k nodes/column, 0 => DVE rows
        self.classes = []                       # [(S, size, kind)]
        for S, cnt in zip(Svals, cnts):
            S = int(S)
            npc = -(-int(cnt) // N_CORES)
            k = 128 // S
            rpp = -(-npc // 128)
            waste = (128 - k * S) / 128.0
            if (k >= 2 and waste <= PE_WASTE_MAX and k <= KMAX
                    and S * rpp >= PE_MIN_COLS):
                self.classes.append((S, -(-npc // k), k))
            else:
                self.classes.append((S, rpp, 0))

        i = np.arange(nodes.shape[0], dtype=np.int64)
        ci = np.searchsorted(Svals, clsS)
        S_arr = Svals
        size_arr = np.array([sz for _, sz, _ in self.classes], np.int64)
        kind_arr = np.array([kd for _, _, kd in self.classes], np.int64)
        goff = np.zeros(len(Svals), np.int64)
        ooff = np.zeros(len(Svals), np.int64)   # sm cols (DVE classes)
        poff = np.zeros(len(Svals), np.int64)   # pe cols (PE classes)
        go = oo = po = 0
        for idx, (S, sz, kd) in enumerate(self.classes):
            goff[idx] = go
            if kd == 0:
                ooff[idx] = oo
                go += S * sz
                oo += sz
            else:
                poff[idx] = po
                go += sz
                po += sz
        self.gcols, self.rpt, self.pecols = go, oo, po

        core = i % N_CORES
        q = (i - starts[ci]) // N_CORES
        kd = kind_arr[ci]
        self.is_pe = kd > 0
        # DVE placement: partition q//size, row q%size
        # PE placement:  lane q%k -> partition S*lane, column q//k
        sz = size_arr[ci]
        p_dve = q // sz
        r_dve = q % sz
        lane = np.where(kd > 0, q % np.maximum(kd, 1), 0)
        col = np.where(kd > 0, q // np.maximum(kd, 1), 0)
        self.nodes = nodes
        self.core = core
        self.part = np.where(self.is_pe, S_arr[ci] * lane, p_dve)
        self.gcol = goff[ci] + np.where(self.is_pe, col, r_dve * S_arr[ci])
        self.ocol_sm = ooff[ci] + r_dve          # valid where ~is_pe
        self.lane = lane                         # valid where is_pe
        self.ocol_pe = poff[ci] + col            # valid where is_pe


def _plan_pieces(sections):
    """Lay sections side by side; return (GCOLS, RPT, PECOLS, wst_spec,
    chunks). chunks = [(g0, g1, [(S, k, pg0, pg1, o0, o1), ...])] with k=0
    for DVE pieces (o into sm) and k>0 for PE pieces (o into peo)."""
    pieces = []
    wst_spec = []
    gbase = obase = pbase = 0
    for sec in sections:
        go = oo = po = 0
        for S, sz, kd in sec.classes:
            if kd == 0:
                max_rows = max(1, CHUNK_COLS // S)
                r = 0
                while r < sz:
                    rows = min(max_rows, sz - r)
                    pieces.append((S, 0, gbase + go + r * S,
                                   gbase + go + (r + rows) * S,
                                   obase + oo + r, obase + oo + r + rows))
                    r += rows
                go += S * sz
                oo += sz
            else:
                if (S, kd) not in wst_spec:
                    wst_spec.append((S, kd))
                c = 0
                while c < sz:
                    cols = min(CHUNK_COLS, sz - c)
                    pieces.append((S, kd, gbase + go + c,
                                   gbase + go + c + cols,
                                   pbase + po + c, pbase + po + c + cols))
                    c += cols
                go += sz
                po += sz
        gbase += sec.gcols
        obase += sec.rpt
        pbase += sec.pecols
    GCOLS, RPT, PECOLS = gbase, obase, pbase

    chunks = []
    cur, cur_cols = [], 0
    for pc in pieces:
        cols = pc[3] - pc[2]
        if cur and cur_cols + cols > CHUNK_COLS:
            chunks.append(cur)
            cur, cur_cols = [], 0
        cur.append(pc)
        cur_cols += cols
    if cur:
        chunks.append(cur)
    out = [(ch[0][2], ch[-1][3], ch) for ch in chunks]
    # ascending size: small first chunk starts compute quickly; with
    # bufs >= n_chunks every chunk is in flight so DMA never stalls
    out.sort(key=lambda c: c[1] - c[0])
    # carve a tiny DVE lead chunk so the first reduce isn't behind a big DMA
    g0, g1, pcs = out[0]
    if g1 - g0 > 2 * FIRST_CHUNK_COLS and all(p[1] == 0 for p in pcs):
        lead, rest, acc = [], [], 0
        for (S, kd, ig0, ig1, o0, o1) in pcs:
            if acc >= FIRST_CHUNK_COLS:
                rest.append((S, kd, ig0, ig1, o0, o1))
                continue
            cols = ig1 - ig0
            if acc + cols > FIRST_CHUNK_COLS and cols > S:
                rows = max(1, (FIRST_CHUNK_COLS - acc) // S)
                rows = min(rows, cols // S - 1)
                mid = ig0 + rows * S
                lead.append((S, kd, ig0, mid, o0, o0 + rows))
                rest.append((S, kd, mid, ig1, o0 + rows, o1))
                acc += rows * S
            else:
                lead.append((S, kd, ig0, ig1, o0, o1))
                acc += cols
        if lead and rest:
            out = [(lead[0][2], lead[-1][3], lead),
                   (rest[0][2], rest[-1][3], rest)] + out[1:]
    return GCOLS, RPT, PECOLS, tuple(wst_spec), tuple(
        (g0, g1, tuple(pcs)) for g0, g1, pcs in out)


def _build_neff(geom):
    """Chunked grid row-sum kernel (DVE halve+reduce / PE ones-matmul)."""
    from concourse import bacc, mybir, tile

    GCOLS, RPT, PECOLS, wst_spec, chunks = geom
    nc = bacc.Bacc("TRN2", target_bir_lowering=False, debug=False,
                   num_devices=N_CORES, enable_partition_id=False)
    f32, bf16 = mybir.dt.float32, mybir.dt.bfloat16
    add = mybir.AluOpType.add
    X = mybir.AxisListType.X
    g = nc.dram_tensor("g", [128, GCOLS], bf16, kind="ExternalInput")
    sm = nc.dram_tensor("sm", [128, max(RPT, 1)], f32,
                        kind="ExternalOutput")
    WC = sum(k for _, k in wst_spec)
    if PECOLS:
        wst = nc.dram_tensor("wst", [128, WC], bf16, kind="ExternalInput")
        peo = nc.dram_tensor("peo", [KMAX, PECOLS], f32,
                             kind="ExternalOutput")
    wst_off = {}
    off = 0
    for S, k in wst_spec:
        wst_off[(S, k)] = off
        off += k

    kmax = max((k for _, k in wst_spec), default=1)
    Copy = None

    with tile.TileContext(nc) as tc:
        with tc.tile_pool(name="p", bufs=max(2, len(chunks))) as pool, \
             tc.tile_pool(name="h", bufs=3) as hpool, \
             tc.tile_pool(name="ps", bufs=2, space="PSUM") as pspool, \
             tc.tile_pool(name="s", bufs=1) as spool:
            Copy = mybir.ActivationFunctionType.Copy
            sums = spool.tile([128, max(RPT, 1)], f32)
            if PECOLS:
                wtile = spool.tile([128, WC], bf16)
                nc.sync.dma_start(out=wtile[:], in_=wst.ap())
                spe = spool.tile([kmax, PECOLS], f32)
            for (g0, g1, pcs) in chunks:
                t = pool.tile([128, g1 - g0], bf16, tag="g")
                nc.sync.dma_start(out=t[:], in_=g.ap()[:, g0:g1])
                for (S, kd, ig0, ig1, o0, o1) in pcs:
                    if kd > 0:
                        wc0 = wst_off[(S, kd)]
                        for go in range(ig0, ig1, PE_GROUP * MM):
                            gcols = min(PE_GROUP * MM, ig1 - go)
                            pt = pspool.tile([kd, PE_GROUP * MM], f32,
                                             tag="pe")
                            nmm = -(-gcols // MM)
                            for j in range(nmm):
                                c0 = go + j * MM
                                cw = min(MM, ig1 - c0)
                                nc.tensor.matmul(
                                    pt[0:kd, j * MM:j * MM + cw],
                                    wtile[:, wc0:wc0 + kd],
                                    t[:, c0 - g0:c0 - g0 + cw],
                                    start=True, stop=True)
                            oc = o0 + (go - ig0)
                            # PSUM can't be DMA'd; stage via ACT copy
                            nc.scalar.activation(
                                out=spe[0:kd, oc:oc + gcols],
                                in_=pt[0:kd, 0:gcols], func=Copy)
                        continue
                    t3 = t[:, ig0 - g0:ig1 - g0].rearrange(
                        "p (r s) -> p r s", s=S)
                    if ig1 - ig0 >= HALVE_MIN_COLS and S % 2 == 0:
                        S2 = S // 2
                        h = hpool.tile([128, (o1 - o0) * S2], bf16,
                                       tag="h")
                        h3 = h[:].rearrange("p (r s) -> p r s", s=S2)
                        nc.vector.tensor_tensor(
                            out=h3, in0=t3[:, :, 0:S2],
                            in1=t3[:, :, S2:S], op=add)
                        if ig1 - ig0 >= HALVE2_MIN_COLS and S2 % 4 == 0:
                            S4 = S2 // 2
                            h2 = hpool.tile([128, (o1 - o0) * S4], bf16,
                                            tag="h2")
                            h23 = h2[:].rearrange("p (r s) -> p r s",
                                                  s=S4)
                            nc.vector.tensor_tensor(
                                out=h23, in0=h3[:, :, 0:S4],
                                in1=h3[:, :, S4:S2], op=add)
                            nc.vector.tensor_reduce(
                                out=sums[:, o0:o1], in_=h23, axis=X,
                                op=add)
                        else:
                            nc.vector.tensor_reduce(
                                out=sums[:, o0:o1], in_=h3, axis=X,
                                op=add)
                    else:
                        nc.vector.tensor_reduce(
                            out=sums[:, o0:o1], in_=t3, axis=X, op=add)
            nc.sync.dma_start(out=sm.ap(), in_=sums[:])
            if PECOLS:
                nc.sync.dma_start(out=peo.ap()[0:kmax, :], in_=spe[:])
    nc.compile()
    return nc


def _run(geom, grids, wst_np):
    """-> (sm [N_CORES,128,RPT], peo [N_CORES,KMAX,PECOLS] or None)."""
    from concourse import bass_utils

    if geom not in _NEFF_CACHE:
        _NEFF_CACHE[geom] = _build_neff(geom)
    nc = _NEFF_CACHE[geom]
    PECOLS = geom[2]
    in_maps = []
    for c in range(N_CORES):
        m = {"g": grids[c]}
        if PECOLS:
            m["wst"] = wst_np
        in_maps.append(m)
    res = bass_utils.run_bass_kernel_spmd(nc, in_maps,
                                          core_ids=list(range(N_CORES)))
    smo = np.stack([res.results[c]["sm"] for c in range(N_CORES)])
    peo = (np.stack([res.results[c]["peo"] for c in range(N_CORES)])
           if PECOLS else None)
    return smo, peo


def _build_wst(wst_spec):
    from ml_dtypes import bfloat16

    WC = sum(k for _, k in wst_spec)
    w = np.zeros((128, max(WC, 1)), bfloat16)
    off = 0
    for S, k in wst_spec:
        for m in range(k):
            w[m * S:(m + 1) * S, off + m] = 1.0
        off += k
    return w


def _flats(sec, GCOLS, RPT, PECOLS):
    """Per-active-node flat indices: grid base, slot stride, output."""
    gflat = (sec.core * 128 + sec.part) * GCOLS + sec.gcol
    gmul = np.where(sec.is_pe, GCOLS, 1).astype(np.int64)
    oflat = np.where(
        sec.is_pe,
        (sec.core * KMAX + sec.lane) * max(PECOLS, 1) + sec.ocol_pe,
        (sec.core * 128 + sec.part) * max(RPT, 1) + sec.ocol_sm)
    return gflat, gmul, oflat


def _gather_sums(sec, oflat, smo, peo):
    out = np.empty(sec.nodes.shape[0], np.float32)
    pe = sec.is_pe
    out[~pe] = smo.reshape(-1)[oflat[~pe]]
    if pe.any():
        out[pe] = peo.reshape(-1)[oflat[pe]]
    return out


def kernel(x, edge_index, W1, b1, W2, b2):
    from ml_dtypes import bfloat16

    x = np.asarray(x, dtype=np.float32)
    W1 = np.asarray(W1, dtype=np.float32).reshape(-1)   # [4] (C_in == 1)
    b1 = np.asarray(b1, dtype=np.float32).reshape(-1)
    W2 = np.asarray(W2, dtype=np.float32)               # [4, 4]
    b2 = np.asarray(b2, dtype=np.float32).reshape(-1)
    ei = np.asarray(edge_index)
    N = x.shape[0]
    E = ei.shape[1]
    assert x.shape[1] == 1 and W1.shape[0] == 4 and W2.shape == (4, 4)
    # b1 == 0 is load-bearing for the s+/s- collapse (spec: fill zeros).
    assert np.all(b1 == 0.0), "kernel specialized to b1 == 0"

    src = ei[0].astype(np.int64)
    dst = ei[1].astype(np.int64)

    # ---- shared host index work ----
    indeg = np.bincount(dst, minlength=N).astype(np.int64)
    slots = indeg + 1                                   # + self slot
    dinv = (1.0 / np.sqrt(slots.astype(np.float32))).astype(np.float32)
    xprime = (x[:, 0] * dinv).astype(np.float32)

    ptr = np.zeros(N + 1, np.int64)
    np.cumsum(indeg, out=ptr[1:])
    es = np.argsort(dst, kind="stable")
    sdst = dst[es]
    ssrc = src[es]
    rank = np.arange(E, dtype=np.int64) - ptr[sdst]

    # ---- layer 1: one section keyed by slots ----
    secA = _Section(slots)
    geomA = _plan_pieces([secA])
    GC_A, RPT_A, PEC_A = geomA[0], geomA[1], geomA[2]
    gfA, gmA, ofA = _flats(secA, GC_A, RPT_A, PEC_A)
    gflatA = np.zeros(N, np.int64)
    gmulA = np.ones(N, np.int64)
    gflatA[secA.nodes] = gfA
    gmulA[secA.nodes] = gmA

    GA = np.zeros(N_CORES * 128 * GC_A, bfloat16)
    xb = xprime.astype(bfloat16)
    GA[gflatA[sdst] + rank * gmulA[sdst]] = xb[ssrc]
    GA[gflatA + indeg * gmulA] = xb                     # self slot (last)

    wstA = _build_wst(geomA[3])
    smA, peA = _run(geomA, GA.reshape(N_CORES, 128, GC_A), wstA)
    sumsA = _gather_sums(secA, ofA, smA, peA)           # section order
    y1p = np.zeros(N, np.float32)
    y1p[secA.nodes] = sumsA
    y1p *= dinv * dinv                                  # y1' = d2 * sum

    # ---- layer 2: sign-split sections ----
    y1b = y1p.astype(bfloat16)
    mB = y1b[ssrc]                                      # per-edge message
    q = (mB > 0)
    posb = np.bincount(sdst[q], minlength=N).astype(np.int64)
    self_pos = (y1b >= 0)                               # zeros -> P grid
    pslots = posb + self_pos
    mslots = (indeg - posb) + (~self_pos)
    excl = np.cumsum(q.astype(np.int64)) - q            # positives before e
    rank_pos = excl - excl[ptr[sdst]]                   # ...within segment
    rank_neg = rank - rank_pos

    secP = _Section(pslots)
    secM = _Section(mslots)
    geomB = _plan_pieces([secP, secM])
    GC_B, RPT_B, PEC_B = geomB[0], geomB[1], geomB[2]
    gfP, gmP, ofP = _flats(secP, GC_B, RPT_B, PEC_B)
    gfM, gmM, ofM = _flats(secM, GC_B, RPT_B, PEC_B)
    # M section sits after P inside the shared tensors
    gfM = gfM + secP.gcols
    ofM = ofM + np.where(secM.is_pe, secP.pecols, secP.rpt)

    gflatP = np.zeros(N, np.int64)
    gmulP = np.ones(N, np.int64)
    gflatP[secP.nodes] = gfP
    gmulP[secP.nodes] = gmP
    gflatM = np.zeros(N, np.int64)
    gmulM = np.ones(N, np.int64)
    gflatM[secM.nodes] = gfM
    gmulM[secM.nodes] = gmM

    GB = np.zeros(N_CORES * 128 * GC_B, bfloat16)
    fe = np.where(q, gflatP[sdst] + rank_pos * gmulP[sdst],
                  gflatM[sdst] + rank_neg * gmulM[sdst])
    GB[fe] = mB
    fs = np.where(self_pos, gflatP + posb * gmulP,
                  gflatM + (indeg - posb) * gmulM)
    GB[fs] = y1b

    wstB = _build_wst(geomB[3])
    smB, peB = _run(geomB, GB.reshape(N_CORES, 128, GC_B), wstB)
    sp = np.zeros(N, np.float32)
    smv = np.zeros(N, np.float32)
    sp[secP.nodes] = _gather_sums(secP, ofP, smB, peB)
    smv[secM.nodes] = _gather_sums(secM, ofM, smB, peB)

    # ---- O(N) host finalize ----
    aj = (np.maximum(W1, 0.0) @ W2).astype(np.float32)  # [4]
    cj = (np.minimum(W1, 0.0) @ W2).astype(np.float32)
    out = (dinv[:, None] *
           (sp[:, None] * aj[None, :] + smv[:, None] * cj[None, :]) +
           b2[None, :])
    return np.ascontiguousarray(out, dtype=np.float32)


# revision 22
# speedup vs baseline: 1.2009x; 1.2009x over previous
"""GCN (2-layer, PyG GCNConv semantics) on 8 Trainium2 NeuronCores.

Strategy (v7)
-------------
Host does layout + O(N) normalization; each NeuronCore reduces dense
bf16 message grids. See v3 notes: layer-1 grid holds x'(u)=dinv(u)*x(u)
(+ self slot); layer-2 messages y1'(u)=dinv(u)*y1(u) are sign-split into
positive/negative grids so s+/s- are plain row-sums (b1 == 0, C_in == 1);
out[v,j] = dinv_v*(a_j*s+ + c_j*s-) + b2_j is O(N) host work.

Row-sums are computed two ways, picked per degree class:
 - DVE path: [128, rows, S] row-major layout; bf16 pairwise-add halving
   (DVE 2x mode) then a 1x tensor_reduce.
 - PE path (classes where k=floor(128/S) wastes <=7% of partitions):
   slot-major layout [S*lane + s, node//k]; row-sum = ones-blockdiag
   [128,k] matmul on the otherwise-idle tensor engine, accumulated in
   PSUM (4 banks per group) and DMA'd straight to DRAM.

Nodes are width-sorted and dealt round-robin to the 8 cores per grid, so
all cores share one SPMD NEFF and work balances to <0.1%; no collectives.
"""
import sys

sys.path.insert(0, "/opt/trn_rl_repo")

import numpy as np

N_CORES = 8
CLS_STEP = 4            # slot-count class granularity
CHUNK_COLS = 4608       # <=9.2KB/partition bf16 per DMA chunk
HALVE_MIN_COLS = 768    # min piece cols for the 2x pairwise-add pass
HALVE2_MIN_COLS = 3072  # min cols for a second halving level (S%8==0)
FIRST_CHUNK_COLS = 384  # tiny lead chunk so the DVE starts ASAP
PE_ENABLE = False       # PE matmul row-sum measured ~0.85 col/ns unramped
                        # (ldweights + PSUM-stage overhead) — loses to DVE
PE_WASTE_MAX = 0.07     # max idle-partition fraction for the PE path
PE_MIN_COLS = 384       # min class cols for the PE path
MM = 512                # matmul moving-operand max free dim
PE_GROUP = 4            # matmuls (PSUM banks) per PSUM tile / out DMA
KMAX = 16               # partition dim of the PE output DRAM tensor

_NEFF_CACHE: dict = {}


class _Section:
    """Packing of one grid section (one slot-count distribution)."""

    def __init__(self, slot_counts):
        active = np.flatnonzero(slot_counts > 0)
        sc = slot_counts[active]
        order = np.argsort(sc, kind="stable")
        nodes = active[order]                   # width-sorted node ids
        widths = sc[order]
        clsS = (CLS_STEP * np.ceil(widths / CLS_STEP)).astype(np.int64)
        Svals, starts, cnts = np.unique(clsS, return_index=True,
                                        return_counts=True)
        # per class: kind k>0 => PE with k nodes/column, 0 => DVE rows
        self.classes = []                       # [(S, size, kind)]
        for S, cnt in zip(Svals, cnts):
            S = int(S)
            npc = -(-int(cnt) // N_CORES)
            k = 128 // S
            rpp = -(-npc // 128)
            waste = (128 - k * S) / 128.0
            if (PE_ENABLE and k >= 2 and waste <= PE_WASTE_MAX
                    and k <= KMAX and S * rpp >= PE_MIN_COLS):
                self.classes.append((S, -(-npc // k), k))
            else:
                self.classes.append((S, rpp, 0))

        i = np.arange(nodes.shape[0], dtype=np.int64)
        ci = np.searchsorted(Svals, clsS)
        S_arr = Svals
        size_arr = np.array([sz for _, sz, _ in self.classes], np.int64)
        kind_arr = np.array([kd for _, _, kd in self.classes], np.int64)
        goff = np.zeros(len(Svals), np.int64)
        ooff = np.zeros(len(Svals), np.int64)   # sm cols (DVE classes)
        poff = np.zeros(len(Svals), np.int64)   # pe cols (PE classes)
        go = oo = po = 0
        for idx, (S, sz, kd) in enumerate(self.classes):
            goff[idx] = go
            if kd == 0:
                ooff[idx] = oo
                go += S * sz
                oo += sz
            else:
                poff[idx] = po
                go += sz
                po += sz
        self.gcols, self.rpt, self.pecols = go, oo, po

        core = i % N_CORES
        q = (i - starts[ci]) // N_CORES
        kd = kind_arr[ci]
        self.is_pe = kd > 0
        # DVE placement: partition q//size, row q%size
        # PE placement:  lane q%k -> partition S*lane, column q//k
        sz = size_arr[ci]
        p_dve = q // sz
        r_dve = q % sz
        lane = np.where(kd > 0, q % np.maximum(kd, 1), 0)
        col = np.where(kd > 0, q // np.maximum(kd, 1), 0)
        self.nodes = nodes
        self.core = core
        self.part = np.where(self.is_pe, S_arr[ci] * lane, p_dve)
        self.gcol = goff[ci] + np.where(self.is_pe, col, r_dve * S_arr[ci])
        self.ocol_sm = ooff[ci] + r_dve          # valid where ~is_pe
        self.lane = lane                         # valid where is_pe
        self.ocol_pe = poff[ci] + col            # valid where is_pe


def _plan_pieces(sections):
    """Lay sections side by side; return (GCOLS, RPT, PECOLS, wst_spec,
    chunks). chunks = [(g0, g1, [(S, k, pg0, pg1, o0, o1), ...])] with k=0
    for DVE pieces (o into sm) and k>0 for PE pieces (o into peo)."""
    pieces = []
    wst_spec = []
    gbase = obase = pbase = 0
    for sec in sections:
        go = oo = po = 0
        for S, sz, kd in sec.classes:
            if kd == 0:
                max_rows = max(1, CHUNK_COLS // S)
                r = 0
                while r < sz:
                    rows = min(max_rows, sz - r)
                    pieces.append((S, 0, gbase + go + r * S,
                                   gbase + go + (r + rows) * S,
                                   obase + oo + r, obase + oo + r + rows))
                    r += rows
                go += S * sz
                oo += sz
            else:
                if (S, kd) not in wst_spec:
                    wst_spec.append((S, kd))
                c = 0
                while c < sz:
                    cols = min(CHUNK_COLS, sz - c)
                    pieces.append((S, kd, gbase + go + c,
                                   gbase + go + c + cols,
                                   pbase + po + c, pbase + po + c + cols))
                    c += cols
                go += sz
                po += sz
        gbase += sec.gcols
        obase += sec.rpt
        pbase += sec.pecols
    GCOLS, RPT, PECOLS = gbase, obase, pbase

    chunks = []
    cur, cur_cols = [], 0
    for pc in pieces:
        cols = pc[3] - pc[2]
        if cur and cur_cols + cols > CHUNK_COLS:
            chunks.append(cur)
            cur, cur_cols = [], 0
        cur.append(pc)
        cur_cols += cols
    if cur:
        chunks.append(cur)
    out = [(ch[0][2], ch[-1][3], ch) for ch in chunks]
    # ascending size: small first chunk starts compute quickly; with
    # bufs >= n_chunks every chunk is in flight so DMA never stalls
    out.sort(key=lambda c: c[1] - c[0])
    # carve a tiny DVE lead chunk so the first reduce isn't behind a big DMA
    g0, g1, pcs = out[0]
    if g1 - g0 > 2 * FIRST_CHUNK_COLS and all(p[1] == 0 for p in pcs):
        lead, rest, acc = [], [], 0
        for (S, kd, ig0, ig1, o0, o1) in pcs:
            if acc >= FIRST_CHUNK_COLS:
                rest.append((S, kd, ig0, ig1, o0, o1))
                continue
            cols = ig1 - ig0
            if acc + cols > FIRST_CHUNK_COLS and cols > S:
                rows = max(1, (FIRST_CHUNK_COLS - acc) // S)
                rows = min(rows, cols // S - 1)
                mid = ig0 + rows * S
                lead.append((S, kd, ig0, mid, o0, o0 + rows))
                rest.append((S, kd, mid, ig1, o0 + rows, o1))
                acc += rows * S
            else:
                lead.append((S, kd, ig0, ig1, o0, o1))
                acc += cols
        if lead and rest:
            out = [(lead[0][2], lead[-1][3], lead),
                   (rest[0][2], rest[-1][3], rest)] + out[1:]
    return GCOLS, RPT, PECOLS, tuple(wst_spec), tuple(
        (g0, g1, tuple(pcs)) for g0, g1, pcs in out)


def _build_neff(geom):
    """Chunked grid row-sum kernel (DVE halve+reduce / PE ones-matmul)."""
    from concourse import bacc, mybir, tile

    GCOLS, RPT, PECOLS, wst_spec, chunks = geom
    nc = bacc.Bacc("TRN2", target_bir_lowering=False, debug=False,
                   num_devices=N_CORES, enable_partition_id=False)
    f32, bf16 = mybir.dt.float32, mybir.dt.bfloat16
    add = mybir.AluOpType.add
    X = mybir.AxisListType.X
    g = nc.dram_tensor("g", [128, GCOLS], bf16, kind="ExternalInput")
    sm = nc.dram_tensor("sm", [128, max(RPT, 1)], f32,
                        kind="ExternalOutput")
    WC = sum(k for _, k in wst_spec)
    if PECOLS:
        wst = nc.dram_tensor("wst", [128, WC], bf16, kind="ExternalInput")
        peo = nc.dram_tensor("peo", [KMAX, PECOLS], f32,
                             kind="ExternalOutput")
    wst_off = {}
    off = 0
    for S, k in wst_spec:
        wst_off[(S, k)] = off
        off += k

    kmax = max((k for _, k in wst_spec), default=1)
    Copy = None

    with tile.TileContext(nc) as tc:
        with tc.tile_pool(name="p", bufs=max(2, len(chunks))) as pool, \
             tc.tile_pool(name="h", bufs=3) as hpool, \
             tc.tile_pool(name="ps", bufs=2, space="PSUM") as pspool, \
             tc.tile_pool(name="s", bufs=1) as spool:
            Copy = mybir.ActivationFunctionType.Copy
            sums = spool.tile([128, max(RPT, 1)], f32)
            if PECOLS:
                wtile = spool.tile([128, WC], bf16)
                nc.sync.dma_start(out=wtile[:], in_=wst.ap())
                spe = spool.tile([kmax, PECOLS], f32)
            for ci_, (g0, g1, pcs) in enumerate(chunks):
                t = pool.tile([128, g1 - g0], bf16, tag="g")
                # alternate the two HWDGE rings (sync / scalar)
                dmaeng = nc.sync if ci_ % 2 == 0 else nc.scalar
                dmaeng.dma_start(out=t[:], in_=g.ap()[:, g0:g1])
                for (S, kd, ig0, ig1, o0, o1) in pcs:
                    if kd > 0:
                        wc0 = wst_off[(S, kd)]
                        for go in range(ig0, ig1, PE_GROUP * MM):
                            gcols = min(PE_GROUP * MM, ig1 - go)
                            pt = pspool.tile([kd, PE_GROUP * MM], f32,
                                             tag="pe")
                            nmm = -(-gcols // MM)
                            for j in range(nmm):
                                c0 = go + j * MM
                                cw = min(MM, ig1 - c0)
                                nc.tensor.matmul(
                                    pt[0:kd, j * MM:j * MM + cw],
                                    wtile[:, wc0:wc0 + kd],
                                    t[:, c0 - g0:c0 - g0 + cw],
                                    start=True, stop=True)
                            oc = o0 + (go - ig0)
                            # PSUM can't be DMA'd; stage via ACT copy
                            nc.scalar.activation(
                                out=spe[0:kd, oc:oc + gcols],
                                in_=pt[0:kd, 0:gcols], func=Copy)
                        continue
                    t3 = t[:, ig0 - g0:ig1 - g0].rearrange(
                        "p (r s) -> p r s", s=S)
                    if ig1 - ig0 >= HALVE_MIN_COLS and S % 2 == 0:
                        S2 = S // 2
                        h = hpool.tile([128, (o1 - o0) * S2], bf16,
                                       tag="h")
                        h3 = h[:].rearrange("p (r s) -> p r s", s=S2)
                        nc.vector.tensor_tensor(
                            out=h3, in0=t3[:, :, 0:S2],
                            in1=t3[:, :, S2:S], op=add)
                        if ig1 - ig0 >= HALVE2_MIN_COLS and S2 % 4 == 0:
                            S4 = S2 // 2
                            h2 = hpool.tile([128, (o1 - o0) * S4], bf16,
                                            tag="h2")
                            h23 = h2[:].rearrange("p (r s) -> p r s",
                                                  s=S4)
                            nc.vector.tensor_tensor(
                                out=h23, in0=h3[:, :, 0:S4],
                                in1=h3[:, :, S4:S2], op=add)
                            nc.vector.tensor_reduce(
                                out=sums[:, o0:o1], in_=h23, axis=X,
                                op=add)
                        else:
                            nc.vector.tensor_reduce(
                                out=sums[:, o0:o1], in_=h3, axis=X,
                                op=add)
                    else:
                        nc.vector.tensor_reduce(
                            out=sums[:, o0:o1], in_=t3, axis=X, op=add)
            nc.sync.dma_start(out=sm.ap(), in_=sums[:])
            if PECOLS:
                nc.sync.dma_start(out=peo.ap()[0:kmax, :], in_=spe[:])
    nc.compile()
    return nc


def _run(geom, grids, wst_np):
    """-> (sm [N_CORES,128,RPT], peo [N_CORES,KMAX,PECOLS] or None)."""
    from concourse import bass_utils

    if geom not in _NEFF_CACHE:
        _NEFF_CACHE[geom] = _build_neff(geom)
    nc = _NEFF_CACHE[geom]
    PECOLS = geom[2]
    in_maps = []
    for c in range(N_CORES):
        m = {"g": grids[c]}
        if PECOLS:
            m["wst"] = wst_np
        in_maps.append(m)
    res = bass_utils.run_bass_kernel_spmd(nc, in_maps,
                                          core_ids=list(range(N_CORES)))
    smo = np.stack([res.results[c]["sm"] for c in range(N_CORES)])
    peo = (np.stack([res.results[c]["peo"] for c in range(N_CORES)])
           if PECOLS else None)
    return smo, peo


def _build_wst(wst_spec):
    from ml_dtypes import bfloat16

    WC = sum(k for _, k in wst_spec)
    w = np.zeros((128, max(WC, 1)), bfloat16)
    off = 0
    for S, k in wst_spec:
        for m in range(k):
            w[m * S:(m + 1) * S, off + m] = 1.0
        off += k
    return w


def _flats(sec, GCOLS, RPT, PECOLS):
    """Per-active-node flat indices: grid base, slot stride, output."""
    gflat = (sec.core * 128 + sec.part) * GCOLS + sec.gcol
    gmul = np.where(sec.is_pe, GCOLS, 1).astype(np.int64)
    oflat = np.where(
        sec.is_pe,
        (sec.core * KMAX + sec.lane) * max(PECOLS, 1) + sec.ocol_pe,
        (sec.core * 128 + sec.part) * max(RPT, 1) + sec.ocol_sm)
    return gflat, gmul, oflat


def _gather_sums(sec, oflat, smo, peo):
    out = np.empty(sec.nodes.shape[0], np.float32)
    pe = sec.is_pe
    out[~pe] = smo.reshape(-1)[oflat[~pe]]
    if pe.any():
        out[pe] = peo.reshape(-1)[oflat[pe]]
    return out


def kernel(x, edge_index, W1, b1, W2, b2):
    from ml_dtypes import bfloat16

    x = np.asarray(x, dtype=np.float32)
    W1 = np.asarray(W1, dtype=np.float32).reshape(-1)   # [4] (C_in == 1)
    b1 = np.asarray(b1, dtype=np.float32).reshape(-1)
    W2 = np.asarray(W2, dtype=np.float32)               # [4, 4]
    b2 = np.asarray(b2, dtype=np.float32).reshape(-1)
    ei = np.asarray(edge_index)
    N = x.shape[0]
    E = ei.shape[1]
    assert x.shape[1] == 1 and W1.shape[0] == 4 and W2.shape == (4, 4)
    # b1 == 0 is load-bearing for the s+/s- collapse (spec: fill zeros).
    assert np.all(b1 == 0.0), "kernel specialized to b1 == 0"

    src = ei[0].astype(np.int64)
    dst = ei[1].astype(np.int64)

    # ---- shared host index work ----
    indeg = np.bincount(dst, minlength=N).astype(np.int64)
    slots = indeg + 1                                   # + self slot
    dinv = (1.0 / np.sqrt(slots.astype(np.float32))).astype(np.float32)
    xprime = (x[:, 0] * dinv).astype(np.float32)

    ptr = np.zeros(N + 1, np.int64)
    np.cumsum(indeg, out=ptr[1:])
    es = np.argsort(dst, kind="stable")
    sdst = dst[es]
    ssrc = src[es]
    rank = np.arange(E, dtype=np.int64) - ptr[sdst]

    # ---- layer 1: one section keyed by slots ----
    secA = _Section(slots)
    geomA = _plan_pieces([secA])
    GC_A, RPT_A, PEC_A = geomA[0], geomA[1], geomA[2]
    gfA, gmA, ofA = _flats(secA, GC_A, RPT_A, PEC_A)
    gflatA = np.zeros(N, np.int64)
    gmulA = np.ones(N, np.int64)
    gflatA[secA.nodes] = gfA
    gmulA[secA.nodes] = gmA

    GA = np.zeros(N_CORES * 128 * GC_A, bfloat16)
    xb = xprime.astype(bfloat16)
    GA[gflatA[sdst] + rank * gmulA[sdst]] = xb[ssrc]
    GA[gflatA + indeg * gmulA] = xb                     # self slot (last)

    wstA = _build_wst(geomA[3])
    smA, peA = _run(geomA, GA.reshape(N_CORES, 128, GC_A), wstA)
    sumsA = _gather_sums(secA, ofA, smA, peA)           # section order
    y1p = np.zeros(N, np.float32)
    y1p[secA.nodes] = sumsA
    y1p *= dinv * dinv                                  # y1' = d2 * sum

    # ---- layer 2: sign-split sections ----
    y1b = y1p.astype(bfloat16)
    mB = y1b[ssrc]                                      # per-edge message
    q = (mB > 0)
    posb = np.bincount(sdst[q], minlength=N).astype(np.int64)
    self_pos = (y1b >= 0)                               # zeros -> P grid
    pslots = posb + self_pos
    mslots = (indeg - posb) + (~self_pos)
    excl = np.cumsum(q.astype(np.int64)) - q            # positives before e
    rank_pos = excl - excl[ptr[sdst]]                   # ...within segment
    rank_neg = rank - rank_pos

    # one section over 2N virtual rows: v < N => P-row of v, else M-row.
    # Same-width P and M rows share degree classes, halving piece count.
    secB = _Section(np.concatenate([pslots, mslots]))
    geomB = _plan_pieces([secB])
    GC_B, RPT_B, PEC_B = geomB[0], geomB[1], geomB[2]
    gfB, gmB, ofB = _flats(secB, GC_B, RPT_B, PEC_B)
    gflatB = np.zeros(2 * N, np.int64)
    gmulB = np.ones(2 * N, np.int64)
    gflatB[secB.nodes] = gfB
    gmulB[secB.nodes] = gmB

    GB = np.zeros(N_CORES * 128 * GC_B, bfloat16)
    fe = np.where(q, gflatB[sdst] + rank_pos * gmulB[sdst],
                  gflatB[N + sdst] + rank_neg * gmulB[N + sdst])
    GB[fe] = mB
    vn = np.where(self_pos, np.arange(N), N + np.arange(N))
    fs = gflatB[vn] + np.where(self_pos, posb, indeg - posb) * gmulB[vn]
    GB[fs] = y1b

    wstB = _build_wst(geomB[3])
    smB, peB = _run(geomB, GB.reshape(N_CORES, 128, GC_B), wstB)
    sumsB = np.zeros(2 * N, np.float32)
    sumsB[secB.nodes] = _gather_sums(secB, ofB, smB, peB)
    sp = sumsB[:N]
    smv = sumsB[N:]

    # ---- O(N) host finalize ----
    aj = (np.maximum(W1, 0.0) @ W2).astype(np.float32)  # [4]
    cj = (np.minimum(W1, 0.0) @ W2).astype(np.float32)
    out = (dinv[:, None] *
           (sp[:, None] * aj[None, :] + smv[:, None] * cj[None, :]) +
           b2[None, :])
    return np.ascontiguousarray(out, dtype=np.float32)


# revision 23
# speedup vs baseline: 1.2494x; 1.0404x over previous
"""GCN (2-layer, PyG GCNConv semantics) on 8 Trainium2 NeuronCores.

Strategy (v7)
-------------
Host does layout + O(N) normalization; each NeuronCore reduces dense
bf16 message grids. See v3 notes: layer-1 grid holds x'(u)=dinv(u)*x(u)
(+ self slot); layer-2 messages y1'(u)=dinv(u)*y1(u) are sign-split into
positive/negative grids so s+/s- are plain row-sums (b1 == 0, C_in == 1);
out[v,j] = dinv_v*(a_j*s+ + c_j*s-) + b2_j is O(N) host work.

Row-sums are computed two ways, picked per degree class:
 - DVE path: [128, rows, S] row-major layout; bf16 pairwise-add halving
   (DVE 2x mode) then a 1x tensor_reduce.
 - PE path (classes where k=floor(128/S) wastes <=7% of partitions):
   slot-major layout [S*lane + s, node//k]; row-sum = ones-blockdiag
   [128,k] matmul on the otherwise-idle tensor engine, accumulated in
   PSUM (4 banks per group) and DMA'd straight to DRAM.

Nodes are width-sorted and dealt round-robin to the 8 cores per grid, so
all cores share one SPMD NEFF and work balances to <0.1%; no collectives.
"""
import sys

sys.path.insert(0, "/opt/trn_rl_repo")

import numpy as np

N_CORES = 8
CLS_STEP = 4            # slot-count class granularity
CHUNK_COLS = 4608       # <=9.2KB/partition bf16 per DMA chunk
HALVE_MIN_COLS = 768    # min piece cols for the 2x pairwise-add pass
HALVE2_MIN_COLS = 3072  # min cols for a second halving level (S%8==0)
FIRST_CHUNK_COLS = 384  # tiny lead chunk so the DVE starts ASAP
PE_ENABLE = False       # PE matmul row-sum measured ~0.85 col/ns unramped
                        # (ldweights + PSUM-stage overhead) — loses to DVE
PE_WASTE_MAX = 0.07     # max idle-partition fraction for the PE path
PE_MIN_COLS = 384       # min class cols for the PE path
MM = 512                # matmul moving-operand max free dim
PE_GROUP = 4            # matmuls (PSUM banks) per PSUM tile / out DMA
KMAX = 16               # partition dim of the PE output DRAM tensor

_NEFF_CACHE: dict = {}


class _Section:
    """Packing of one grid section (one slot-count distribution)."""

    def __init__(self, slot_counts):
        active = np.flatnonzero(slot_counts > 0)
        sc = slot_counts[active]
        order = np.argsort(sc, kind="stable")
        nodes = active[order]                   # width-sorted node ids
        widths = sc[order]
        clsS = (CLS_STEP * np.ceil(widths / CLS_STEP)).astype(np.int64)
        Svals, starts, cnts = np.unique(clsS, return_index=True,
                                        return_counts=True)
        # per class: kind k>0 => PE with k nodes/column, 0 => DVE rows
        self.classes = []                       # [(S, size, kind)]
        for S, cnt in zip(Svals, cnts):
            S = int(S)
            npc = -(-int(cnt) // N_CORES)
            k = 128 // S
            rpp = -(-npc // 128)
            waste = (128 - k * S) / 128.0
            if (PE_ENABLE and k >= 2 and waste <= PE_WASTE_MAX
                    and k <= KMAX and S * rpp >= PE_MIN_COLS):
                self.classes.append((S, -(-npc // k), k))
            else:
                self.classes.append((S, rpp, 0))

        i = np.arange(nodes.shape[0], dtype=np.int64)
        ci = np.searchsorted(Svals, clsS)
        S_arr = Svals
        size_arr = np.array([sz for _, sz, _ in self.classes], np.int64)
        kind_arr = np.array([kd for _, _, kd in self.classes], np.int64)
        goff = np.zeros(len(Svals), np.int64)
        ooff = np.zeros(len(Svals), np.int64)   # sm cols (DVE classes)
        poff = np.zeros(len(Svals), np.int64)   # pe cols (PE classes)
        go = oo = po = 0
        for idx, (S, sz, kd) in enumerate(self.classes):
            goff[idx] = go
            if kd == 0:
                ooff[idx] = oo
                go += S * sz
                oo += sz
            else:
                poff[idx] = po
                go += sz
                po += sz
        self.gcols, self.rpt, self.pecols = go, oo, po

        core = i % N_CORES
        q = (i - starts[ci]) // N_CORES
        kd = kind_arr[ci]
        self.is_pe = kd > 0
        # DVE placement: partition q//size, row q%size
        # PE placement:  lane q%k -> partition S*lane, column q//k
        sz = size_arr[ci]
        p_dve = q // sz
        r_dve = q % sz
        lane = np.where(kd > 0, q % np.maximum(kd, 1), 0)
        col = np.where(kd > 0, q // np.maximum(kd, 1), 0)
        self.nodes = nodes
        self.core = core
        self.part = np.where(self.is_pe, S_arr[ci] * lane, p_dve)
        self.gcol = goff[ci] + np.where(self.is_pe, col, r_dve * S_arr[ci])
        self.ocol_sm = ooff[ci] + r_dve          # valid where ~is_pe
        self.lane = lane                         # valid where is_pe
        self.ocol_pe = poff[ci] + col            # valid where is_pe


def _plan_pieces(sections):
    """Lay sections side by side; return (GCOLS, RPT, PECOLS, wst_spec,
    chunks). chunks = [(g0, g1, [(S, k, pg0, pg1, o0, o1), ...])] with k=0
    for DVE pieces (o into sm) and k>0 for PE pieces (o into peo)."""
    pieces = []
    wst_spec = []
    gbase = obase = pbase = 0
    for sec in sections:
        go = oo = po = 0
        for S, sz, kd in sec.classes:
            if kd == 0:
                max_rows = max(1, CHUNK_COLS // S)
                r = 0
                while r < sz:
                    rows = min(max_rows, sz - r)
                    pieces.append((S, 0, gbase + go + r * S,
                                   gbase + go + (r + rows) * S,
                                   obase + oo + r, obase + oo + r + rows))
                    r += rows
                go += S * sz
                oo += sz
            else:
                if (S, kd) not in wst_spec:
                    wst_spec.append((S, kd))
                c = 0
                while c < sz:
                    cols = min(CHUNK_COLS, sz - c)
                    pieces.append((S, kd, gbase + go + c,
                                   gbase + go + c + cols,
                                   pbase + po + c, pbase + po + c + cols))
                    c += cols
                go += sz
                po += sz
        gbase += sec.gcols
        obase += sec.rpt
        pbase += sec.pecols
    GCOLS, RPT, PECOLS = gbase, obase, pbase

    chunks = []
    cur, cur_cols = [], 0
    for pc in pieces:
        cols = pc[3] - pc[2]
        if cur and cur_cols + cols > CHUNK_COLS:
            chunks.append(cur)
            cur, cur_cols = [], 0
        cur.append(pc)
        cur_cols += cols
    if cur:
        chunks.append(cur)
    out = [(ch[0][2], ch[-1][3], ch) for ch in chunks]
    # ascending size: small first chunk starts compute quickly; with
    # bufs >= n_chunks every chunk is in flight so DMA never stalls
    out.sort(key=lambda c: c[1] - c[0])
    # carve a tiny DVE lead chunk so the first reduce isn't behind a big DMA
    g0, g1, pcs = out[0]
    if g1 - g0 > 2 * FIRST_CHUNK_COLS and all(p[1] == 0 for p in pcs):
        lead, rest, acc = [], [], 0
        for (S, kd, ig0, ig1, o0, o1) in pcs:
            if acc >= FIRST_CHUNK_COLS:
                rest.append((S, kd, ig0, ig1, o0, o1))
                continue
            cols = ig1 - ig0
            if acc + cols > FIRST_CHUNK_COLS and cols > S:
                rows = max(1, (FIRST_CHUNK_COLS - acc) // S)
                rows = min(rows, cols // S - 1)
                mid = ig0 + rows * S
                lead.append((S, kd, ig0, mid, o0, o0 + rows))
                rest.append((S, kd, mid, ig1, o0 + rows, o1))
                acc += rows * S
            else:
                lead.append((S, kd, ig0, ig1, o0, o1))
                acc += cols
        if lead and rest:
            out = [(lead[0][2], lead[-1][3], lead),
                   (rest[0][2], rest[-1][3], rest)] + out[1:]
    return GCOLS, RPT, PECOLS, tuple(wst_spec), tuple(
        (g0, g1, tuple(pcs)) for g0, g1, pcs in out)


def _build_neff(geom):
    """Chunked grid row-sum kernel (DVE halve+reduce / PE ones-matmul)."""
    from concourse import bacc, mybir, tile

    GCOLS, RPT, PECOLS, wst_spec, chunks = geom
    nc = bacc.Bacc("TRN2", target_bir_lowering=False, debug=False,
                   num_devices=N_CORES, enable_partition_id=False)
    f32, bf16 = mybir.dt.float32, mybir.dt.bfloat16
    add = mybir.AluOpType.add
    X = mybir.AxisListType.X
    g = nc.dram_tensor("g", [128, GCOLS], bf16, kind="ExternalInput")
    sm = nc.dram_tensor("sm", [128, max(RPT, 1)], f32,
                        kind="ExternalOutput")
    WC = sum(k for _, k in wst_spec)
    if PECOLS:
        wst = nc.dram_tensor("wst", [128, WC], bf16, kind="ExternalInput")
        peo = nc.dram_tensor("peo", [KMAX, PECOLS], f32,
                             kind="ExternalOutput")
    wst_off = {}
    off = 0
    for S, k in wst_spec:
        wst_off[(S, k)] = off
        off += k

    kmax = max((k for _, k in wst_spec), default=1)
    Copy = None

    with tile.TileContext(nc) as tc:
        with tc.tile_pool(name="p", bufs=max(2, len(chunks))) as pool, \
             tc.tile_pool(name="h", bufs=3) as hpool, \
             tc.tile_pool(name="ps", bufs=2, space="PSUM") as pspool, \
             tc.tile_pool(name="s", bufs=1) as spool:
            Copy = mybir.ActivationFunctionType.Copy
            sums = spool.tile([128, max(RPT, 1)], f32)
            if PECOLS:
                wtile = spool.tile([128, WC], bf16)
                nc.sync.dma_start(out=wtile[:], in_=wst.ap())
                spe = spool.tile([kmax, PECOLS], f32)
            for (g0, g1, pcs) in chunks:
                t = pool.tile([128, g1 - g0], bf16, tag="g")
                nc.sync.dma_start(out=t[:], in_=g.ap()[:, g0:g1])
                for (S, kd, ig0, ig1, o0, o1) in pcs:
                    if kd > 0:
                        wc0 = wst_off[(S, kd)]
                        for go in range(ig0, ig1, PE_GROUP * MM):
                            gcols = min(PE_GROUP * MM, ig1 - go)
                            pt = pspool.tile([kd, PE_GROUP * MM], f32,
                                             tag="pe")
                            nmm = -(-gcols // MM)
                            for j in range(nmm):
                                c0 = go + j * MM
                                cw = min(MM, ig1 - c0)
                                nc.tensor.matmul(
                                    pt[0:kd, j * MM:j * MM + cw],
                                    wtile[:, wc0:wc0 + kd],
                                    t[:, c0 - g0:c0 - g0 + cw],
                                    start=True, stop=True)
                            oc = o0 + (go - ig0)
                            # PSUM can't be DMA'd; stage via ACT copy
                            nc.scalar.activation(
                                out=spe[0:kd, oc:oc + gcols],
                                in_=pt[0:kd, 0:gcols], func=Copy)
                        continue
                    t3 = t[:, ig0 - g0:ig1 - g0].rearrange(
                        "p (r s) -> p r s", s=S)
                    if ig1 - ig0 >= HALVE_MIN_COLS and S % 2 == 0:
                        S2 = S // 2
                        h = hpool.tile([128, (o1 - o0) * S2], bf16,
                                       tag="h")
                        h3 = h[:].rearrange("p (r s) -> p r s", s=S2)
                        nc.vector.tensor_tensor(
                            out=h3, in0=t3[:, :, 0:S2],
                            in1=t3[:, :, S2:S], op=add)
                        if ig1 - ig0 >= HALVE2_MIN_COLS and S2 % 4 == 0:
                            S4 = S2 // 2
                            h2 = hpool.tile([128, (o1 - o0) * S4], bf16,
                                            tag="h2")
                            h23 = h2[:].rearrange("p (r s) -> p r s",
                                                  s=S4)
                            nc.vector.tensor_tensor(
                                out=h23, in0=h3[:, :, 0:S4],
                                in1=h3[:, :, S4:S2], op=add)
                            nc.vector.tensor_reduce(
                                out=sums[:, o0:o1], in_=h23, axis=X,
                                op=add)
                        else:
                            nc.vector.tensor_reduce(
                                out=sums[:, o0:o1], in_=h3, axis=X,
                                op=add)
                    else:
                        nc.vector.tensor_reduce(
                            out=sums[:, o0:o1], in_=t3, axis=X, op=add)
            nc.sync.dma_start(out=sm.ap(), in_=sums[:])
            if PECOLS:
                nc.sync.dma_start(out=peo.ap()[0:kmax, :], in_=spe[:])
    nc.compile()
    return nc


def _run(geom, grids, wst_np):
    """-> (sm [N_CORES,128,RPT], peo [N_CORES,KMAX,PECOLS] or None)."""
    from concourse import bass_utils

    if geom not in _NEFF_CACHE:
        _NEFF_CACHE[geom] = _build_neff(geom)
    nc = _NEFF_CACHE[geom]
    PECOLS = geom[2]
    in_maps = []
    for c in range(N_CORES):
        m = {"g": grids[c]}
        if PECOLS:
            m["wst"] = wst_np
        in_maps.append(m)
    res = bass_utils.run_bass_kernel_spmd(nc, in_maps,
                                          core_ids=list(range(N_CORES)))
    smo = np.stack([res.results[c]["sm"] for c in range(N_CORES)])
    peo = (np.stack([res.results[c]["peo"] for c in range(N_CORES)])
           if PECOLS else None)
    return smo, peo


def _build_wst(wst_spec):
    from ml_dtypes import bfloat16

    WC = sum(k for _, k in wst_spec)
    w = np.zeros((128, max(WC, 1)), bfloat16)
    off = 0
    for S, k in wst_spec:
        for m in range(k):
            w[m * S:(m + 1) * S, off + m] = 1.0
        off += k
    return w


def _flats(sec, GCOLS, RPT, PECOLS):
    """Per-active-node flat indices: grid base, slot stride, output."""
    gflat = (sec.core * 128 + sec.part) * GCOLS + sec.gcol
    gmul = np.where(sec.is_pe, GCOLS, 1).astype(np.int64)
    oflat = np.where(
        sec.is_pe,
        (sec.core * KMAX + sec.lane) * max(PECOLS, 1) + sec.ocol_pe,
        (sec.core * 128 + sec.part) * max(RPT, 1) + sec.ocol_sm)
    return gflat, gmul, oflat


def _gather_sums(sec, oflat, smo, peo):
    out = np.empty(sec.nodes.shape[0], np.float32)
    pe = sec.is_pe
    out[~pe] = smo.reshape(-1)[oflat[~pe]]
    if pe.any():
        out[pe] = peo.reshape(-1)[oflat[pe]]
    return out


def kernel(x, edge_index, W1, b1, W2, b2):
    from ml_dtypes import bfloat16

    x = np.asarray(x, dtype=np.float32)
    W1 = np.asarray(W1, dtype=np.float32).reshape(-1)   # [4] (C_in == 1)
    b1 = np.asarray(b1, dtype=np.float32).reshape(-1)
    W2 = np.asarray(W2, dtype=np.float32)               # [4, 4]
    b2 = np.asarray(b2, dtype=np.float32).reshape(-1)
    ei = np.asarray(edge_index)
    N = x.shape[0]
    E = ei.shape[1]
    assert x.shape[1] == 1 and W1.shape[0] == 4 and W2.shape == (4, 4)
    # b1 == 0 is load-bearing for the s+/s- collapse (spec: fill zeros).
    assert np.all(b1 == 0.0), "kernel specialized to b1 == 0"

    src = ei[0].astype(np.int64)
    dst = ei[1].astype(np.int64)

    # ---- shared host index work ----
    indeg = np.bincount(dst, minlength=N).astype(np.int64)
    slots = indeg + 1                                   # + self slot
    dinv = (1.0 / np.sqrt(slots.astype(np.float32))).astype(np.float32)
    xprime = (x[:, 0] * dinv).astype(np.float32)

    ptr = np.zeros(N + 1, np.int64)
    np.cumsum(indeg, out=ptr[1:])
    es = np.argsort(dst, kind="stable")
    sdst = dst[es]
    ssrc = src[es]
    rank = np.arange(E, dtype=np.int64) - ptr[sdst]

    # ---- layer 1: one section keyed by slots ----
    secA = _Section(slots)
    geomA = _plan_pieces([secA])
    GC_A, RPT_A, PEC_A = geomA[0], geomA[1], geomA[2]
    gfA, gmA, ofA = _flats(secA, GC_A, RPT_A, PEC_A)
    gflatA = np.zeros(N, np.int64)
    gmulA = np.ones(N, np.int64)
    gflatA[secA.nodes] = gfA
    gmulA[secA.nodes] = gmA

    GA = np.zeros(N_CORES * 128 * GC_A, bfloat16)
    xb = xprime.astype(bfloat16)
    GA[gflatA[sdst] + rank * gmulA[sdst]] = xb[ssrc]
    GA[gflatA + indeg * gmulA] = xb                     # self slot (last)

    wstA = _build_wst(geomA[3])
    smA, peA = _run(geomA, GA.reshape(N_CORES, 128, GC_A), wstA)
    sumsA = _gather_sums(secA, ofA, smA, peA)           # section order
    y1p = np.zeros(N, np.float32)
    y1p[secA.nodes] = sumsA
    y1p *= dinv * dinv                                  # y1' = d2 * sum

    # ---- layer 2: sign-split sections ----
    y1b = y1p.astype(bfloat16)
    mB = y1b[ssrc]                                      # per-edge message
    q = (mB > 0)
    posb = np.bincount(sdst[q], minlength=N).astype(np.int64)
    self_pos = (y1b >= 0)                               # zeros -> P grid
    pslots = posb + self_pos
    mslots = (indeg - posb) + (~self_pos)
    excl = np.cumsum(q.astype(np.int64)) - q            # positives before e
    rank_pos = excl - excl[ptr[sdst]]                   # ...within segment
    rank_neg = rank - rank_pos

    # one section over 2N virtual rows: v < N => P-row of v, else M-row.
    # Same-width P and M rows share degree classes, halving piece count.
    secB = _Section(np.concatenate([pslots, mslots]))
    geomB = _plan_pieces([secB])
    GC_B, RPT_B, PEC_B = geomB[0], geomB[1], geomB[2]
    gfB, gmB, ofB = _flats(secB, GC_B, RPT_B, PEC_B)
    gflatB = np.zeros(2 * N, np.int64)
    gmulB = np.ones(2 * N, np.int64)
    gflatB[secB.nodes] = gfB
    gmulB[secB.nodes] = gmB

    GB = np.zeros(N_CORES * 128 * GC_B, bfloat16)
    fe = np.where(q, gflatB[sdst] + rank_pos * gmulB[sdst],
                  gflatB[N + sdst] + rank_neg * gmulB[N + sdst])
    GB[fe] = mB
    vn = np.where(self_pos, np.arange(N), N + np.arange(N))
    fs = gflatB[vn] + np.where(self_pos, posb, indeg - posb) * gmulB[vn]
    GB[fs] = y1b

    wstB = _build_wst(geomB[3])
    smB, peB = _run(geomB, GB.reshape(N_CORES, 128, GC_B), wstB)
    sumsB = np.zeros(2 * N, np.float32)
    sumsB[secB.nodes] = _gather_sums(secB, ofB, smB, peB)
    sp = sumsB[:N]
    smv = sumsB[N:]

    # ---- O(N) host finalize ----
    aj = (np.maximum(W1, 0.0) @ W2).astype(np.float32)  # [4]
    cj = (np.minimum(W1, 0.0) @ W2).astype(np.float32)
    out = (dinv[:, None] *
           (sp[:, None] * aj[None, :] + smv[:, None] * cj[None, :]) +
           b2[None, :])
    return np.ascontiguousarray(out, dtype=np.float32)
